# revision 1
# baseline (speedup 1.0000x reference)
"""Context2Query kernel for Trainium2 (Bass/Tile), 8 NeuronCores.

Computes, for inputs u[B, M, D] (query encodings) and s[B, N, M]
(similarity logits):

    A = softmax(s, axis=-1)            # [B, N, M]
    out = einsum('bnm,bmd->bdn', A, u) # [B, D, N]

Sharding: data-parallel over batch. B=16 across 8 cores -> 2 batches/core.
Per batch, per 128-row tile of s (n on partitions):
  - ACT: E = exp(s) in bf16 with fused row-sum (accum_out)  [no max-sub
    needed: logits are N(0,1), exp range ~e^+-6 is safe in fp32]
  - DVE: rinv = 1/sum;  A = E * rinv (per-partition scalar)
  - DMA xbar transpose (SBUF->SBUF, bf16): A tile -> A^T laid out
    [m_in_block(P), t, m_block, n] so the tensor engine sees contraction
    dim m on partitions.
  - PE: out[dblk, n-chunk] += u_bf16[mblk, dblk]^T @ A^T[mblk, n-chunk]
    accumulated over 16 m-blocks in one PSUM bank. PE does only matmuls
    (stays HAM-warm).
u is cast fp32->bf16 for free during its DMA load (SWDGE cast).
"""

import numpy as np

import concourse.bacc as bacc
import concourse.mybir as mybir
import concourse.tile as tile
from concourse.bass_utils import run_bass_kernel_spmd
from concourse.masks import make_identity

F32 = mybir.dt.float32
BF16 = mybir.dt.bfloat16
P = 128

N_CORES = 8


def build_nc(
    B_local,
    N,
    M,
    D,
    n_cores=N_CORES,
    NT=512,
    repeat=1,
    tr_mode="dma",
    db_lim=None,
    u_bf16_host=False,
    skip_tr=False,
    dep_free=False,
    tr_eng="sync",
    mix_pe_ts=(1, 3),
    tr_grouped=False,
    rhs_contig=False,
    sim_trace=False,
):
    assert N % NT == 0 and M % P == 0 and D % P == 0 and NT % P == 0
    assert tr_mode in ("dma", "pe", "mix", "pe2", "hy")
    nc = bacc.Bacc("TRN2", target_bir_lowering=False, num_devices=n_cores)
    s = nc.dram_tensor("s", [B_local, N, M], F32, kind="ExternalInput").ap()
    u_dt = BF16 if u_bf16_host else F32
    u = nc.dram_tensor("u", [B_local, M, D], u_dt, kind="ExternalInput").ap()
    out = nc.dram_tensor("out", [B_local, D, N], F32, kind="ExternalOutput").ap()

    MB = M // P  # contraction blocks
    DB = D // P  # output-partition blocks
    NCH = N // NT  # n chunks
    T = NT // P  # 128-row subtiles per chunk

    with tile.TileContext(nc, trace_sim=sim_trace) as tc:
        with (
            tc.tile_pool(name="u_pool", bufs=2) as u_pool,
            tc.tile_pool(name="s_pool", bufs=4) as s_pool,
            tc.tile_pool(
                name="e_pool",
                bufs=(6 if (tr_grouped or tr_mode in ("pe2", "hy")) else 3),
            ) as e_pool,
            tc.tile_pool(name="at_pool", bufs=2) as at_pool,
            tc.tile_pool(name="o_pool", bufs=2) as o_pool,
            tc.tile_pool(name="st_pool", bufs=4 * T) as st_pool,
            tc.tile_pool(name="singles", bufs=1) as singles,
            tc.tile_pool(name="ps_pool", bufs=4, space="PSUM") as ps_pool,
            tc.tile_pool(name="pst_pool", bufs=2, space="PSUM") as pst_pool,
        ):
            identity = None
            if tr_mode in ("pe", "mix", "pe2", "hy"):
                identity = singles.tile([P, P], BF16)
                make_identity(nc, identity)
            at_const = None
            if dep_free:
                at_const = singles.tile([P, T, MB, P], BF16)
                nc.vector.memset(at_const[:], 0)
            tr_dma = nc.sync if tr_eng == "sync" else nc.scalar
            for b in [b for _ in range(repeat) for b in range(B_local)]:
                u_bf = u_pool.tile([P, MB, D], BF16)
                u_src = u[b].rearrange("(mB p) d -> p mB d", p=P)
                if u_bf16_host:
                    nc.sync.dma_start(out=u_bf[:], in_=u_src)
                else:
                    nc.gpsimd.dma_start(out=u_bf[:], in_=u_src)
                if tr_mode == "hy":
                    # Hybrid pipeline: per chunk, subtiles 0/2 transpose via
                    # DMA xbar (pair issued adjacently to amortize the
                    # xbar-mode switch), subtiles 1/3 via PE in small packets
                    # interleaved between matmul groups.
                    GRP = MB // 2
                    e_cur = {}

                    def produce_e(cc, t):
                        n0 = cc * NT + t * P
                        s_t = s_pool.tile([P, M], F32)
                        nc.sync.dma_start(out=s_t[:], in_=s[b, n0 : n0 + P, :])
                        e_t = e_pool.tile([P, M], BF16)
                        sum_t = st_pool.tile([P, 1], F32, tag="sum")
                        nc.scalar.activation(
                            out=e_t[:],
                            in_=s_t[:],
                            func=mybir.ActivationFunctionType.Exp,
                            accum_out=sum_t[:],
                        )
                        rinv = st_pool.tile([P, 1], F32, tag="rinv")
                        nc.vector.reciprocal(rinv[:], sum_t[:])
                        nc.vector.tensor_scalar_mul(e_t[:], e_t[:], rinv[:])
                        e_cur[(cc, t)] = e_t

                    def pe_tr_half(cc, t, half, AT_next):
                        e_t = e_cur[(cc, t)]
                        ps_t = pst_pool.tile([P, GRP, P], BF16)
                        for k in range(GRP):
                            mblk = half * GRP + k
                            nc.tensor.transpose(
                                ps_t[:, k], e_t[:, mblk * P : (mblk + 1) * P], identity
                            )
                        at_dst = AT_next[:, t, half * GRP : (half + 1) * GRP, :]
                        if half == 0:
                            nc.vector.tensor_copy(out=at_dst, in_=ps_t[:])
                        else:
                            nc.scalar.copy(out=at_dst, in_=ps_t[:])

                    def produce_slot(cc, slot, AT_next):
                        if slot == 0:
                            produce_e(cc, 0)
                        elif slot == 1:
                            produce_e(cc, 1)
                            pe_tr_half(cc, 1, 0, AT_next)
                        elif slot == 2:
                            pe_tr_half(cc, 1, 1, AT_next)
                        elif slot == 3:
                            produce_e(cc, 2)
                            nc.sync.dma_start_transpose(
                                AT_next[:, 0], e_cur[(cc, 0)][:]
                            )
                            nc.sync.dma_start_transpose(
                                AT_next[:, 2], e_cur[(cc, 2)][:]
                            )
                        elif slot == 4:
                            produce_e(cc, 3)
                            pe_tr_half(cc, 3, 0, AT_next)
                        elif slot == 5:
                            pe_tr_half(cc, 3, 1, AT_next)

                    NSLOT = 6
                    ATs = at_pool.tile([P, T, MB, P], BF16)
                    for slot in range(NSLOT):
                        produce_slot(0, slot, ATs)
                    for c in range(NCH):
                        AT_cur = ATs
                        if c + 1 < NCH:
                            ATs = at_pool.tile([P, T, MB, P], BF16)
                        o_t = o_pool.tile([P, DB, NT], F32)
                        for dblk in range(DB):
                            ps = ps_pool.tile([P, NT], F32)
                            for mblk in range(MB):
                                nc.tensor.matmul(
                                    ps[:],
                                    u_bf[:, mblk, dblk * P : (dblk + 1) * P],
                                    AT_cur[:, :, mblk, :],
                                    start=(mblk == 0),
                                    stop=(mblk == MB - 1),
                                )
                            if dblk % 2 == 0:
                                nc.vector.tensor_copy(out=o_t[:, dblk, :], in_=ps[:])
                            else:
                                nc.scalar.copy(out=o_t[:, dblk, :], in_=ps[:])
                            if c + 1 < NCH and dblk < NSLOT:
                                produce_slot(c + 1, dblk, ATs)
                        if c + 1 < NCH:
                            for slot in range(DB, NSLOT):
                                produce_slot(c + 1, slot, ATs)
                        nc.sync.dma_start(
                            out=out[b].rearrange("(dB p) n -> p dB n", p=P)[
                                :, :, c * NT : (c + 1) * NT
                            ],
                            in_=o_t[:],
                        )
                    continue
                if tr_mode == "pe2":
                    # Software-pipelined emission: chunk c+1's softmax +
                    # PE-transposes are emitted in small packets between
                    # chunk c's matmul groups, so PE transpose bursts stay
                    # short (HAM stays warm) and overlap naturally.
                    GRP = MB // 2
                    e_cur = {}

                    def produce_packet(cc, pkt, AT_next):
                        t, half = pkt // 2, pkt % 2
                        if half == 0:
                            n0 = cc * NT + t * P
                            s_t = s_pool.tile([P, M], F32)
                            nc.sync.dma_start(out=s_t[:], in_=s[b, n0 : n0 + P, :])
                            e_t = e_pool.tile([P, M], BF16)
                            sum_t = st_pool.tile([P, 1], F32, tag="sum")
                            nc.scalar.activation(
                                out=e_t[:],
                                in_=s_t[:],
                                func=mybir.ActivationFunctionType.Exp,
                                accum_out=sum_t[:],
                            )
                            rinv = st_pool.tile([P, 1], F32, tag="rinv")
                            nc.vector.reciprocal(rinv[:], sum_t[:])
                            nc.vector.tensor_scalar_mul(e_t[:], e_t[:], rinv[:])
                            e_cur[(cc, t)] = e_t
                        e_t = e_cur[(cc, t)]
                        ps_t = pst_pool.tile([P, GRP, P], BF16)
                        for k in range(GRP):
                            mblk = half * GRP + k
                            nc.tensor.transpose(
                                ps_t[:, k], e_t[:, mblk * P : (mblk + 1) * P], identity
                            )
                        at_dst = AT_next[:, t, half * GRP : (half + 1) * GRP, :]
                        if pkt % 2 == 0:
                            nc.vector.tensor_copy(out=at_dst, in_=ps_t[:])
                        else:
                            nc.scalar.copy(out=at_dst, in_=ps_t[:])

                    ATs = at_pool.tile([P, T, MB, P], BF16)
                    for pkt in range(2 * T):
                        produce_packet(0, pkt, ATs)
                    for c in range(NCH):
                        AT_cur = ATs
                        if c + 1 < NCH:
                            ATs = at_pool.tile([P, T, MB, P], BF16)
                        o_t = o_pool.tile([P, DB, NT], F32)
                        for dblk in range(DB):
                            ps = ps_pool.tile([P, NT], F32)
                            for mblk in range(MB):
                                nc.tensor.matmul(
                                    ps[:],
                                    u_bf[:, mblk, dblk * P : (dblk + 1) * P],
                                    AT_cur[:, :, mblk, :],
                                    start=(mblk == 0),
                                    stop=(mblk == MB - 1),
                                )
                            if dblk % 2 == 0:
                                nc.vector.tensor_copy(out=o_t[:, dblk, :], in_=ps[:])
                            else:
                                nc.scalar.copy(out=o_t[:, dblk, :], in_=ps[:])
                            if c + 1 < NCH and dblk < 2 * T:
                                produce_packet(c + 1, dblk, ATs)
                        if c + 1 < NCH:
                            for pkt in range(DB, 2 * T):
                                produce_packet(c + 1, pkt, ATs)
                        nc.sync.dma_start(
                            out=out[b].rearrange("(dB p) n -> p dB n", p=P)[
                                :, :, c * NT : (c + 1) * NT
                            ],
                            in_=o_t[:],
                        )
                    continue
                for c in range(NCH):
                    AT = at_pool.tile([P, T, MB, P], BF16)
                    if skip_tr:
                        nc.vector.memset(AT[:], 0)
                    e_tiles = {}
                    for t in range(T):
                        if skip_tr:
                            break
                        n0 = c * NT + t * P
                        s_t = s_pool.tile([P, M], F32)
                        nc.sync.dma_start(out=s_t[:], in_=s[b, n0 : n0 + P, :])
                        e_t = e_pool.tile([P, M], BF16)
                        sum_t = st_pool.tile([P, 1], F32, tag="sum")
                        nc.scalar.activation(
                            out=e_t[:],
                            in_=s_t[:],
                            func=mybir.ActivationFunctionType.Exp,
                            accum_out=sum_t[:],
                        )
                        rinv = st_pool.tile([P, 1], F32, tag="rinv")
                        nc.vector.reciprocal(rinv[:], sum_t[:])
                        nc.vector.tensor_scalar_mul(e_t[:], e_t[:], rinv[:])
                        if tr_grouped:
                            e_tiles[t] = e_t
                            continue
                        _emit_transpose(
                            nc, tr_mode, tr_dma, mix_pe_ts, AT, t, e_t,
                            identity, pst_pool, MB,
                        )
                    if tr_grouped and not skip_tr:
                        for t in range(T):
                            _emit_transpose(
                                nc, tr_mode, tr_dma, mix_pe_ts, AT, t,
                                e_tiles[t], identity, pst_pool, MB,
                            )
                    o_t = o_pool.tile([P, DB, NT], F32)
                    for dblk in range(DB if db_lim is None else db_lim):
                        ps = ps_pool.tile([P, NT], F32)
                        rhs_src = at_const if dep_free else AT
                        for mblk in range(MB):
                            if rhs_contig:
                                rhs = rhs_src[:, mblk % T, 0 : NT // P, :]
                            else:
                                rhs = rhs_src[:, :, mblk, :]
                            nc.tensor.matmul(
                                ps[:],
                                u_bf[:, mblk, dblk * P : (dblk + 1) * P],
                                rhs,
                                start=(mblk == 0),
                                stop=(mblk == MB - 1),
                            )
                        nc.any.tensor_copy(out=o_t[:, dblk, :], in_=ps[:])
                    nc.sync.dma_start(
                        out=out[b].rearrange("(dB p) n -> p dB n", p=P)[
                            :, :, c * NT : (c + 1) * NT
                        ],
                        in_=o_t[:],
                    )
    nc.compile()
    return nc


def _emit_transpose(nc, tr_mode, tr_dma, mix_pe_ts, AT, t, e_t, identity, pst_pool, MB):
    use_pe = tr_mode == "pe" or (tr_mode == "mix" and t in mix_pe_ts)
    if not use_pe:
        tr_dma.dma_start_transpose(AT[:, t], e_t[:])
    else:
        # PE transpose in groups of 8 m-blocks per PSUM bank
        # (bf16: 8*128 = 1024 elems = 2 KiB)
        for g in range(MB // 8):
            ps_t = pst_pool.tile([128, 8, 128], BF16)
            for k in range(8):
                mblk = g * 8 + k
                nc.tensor.transpose(
                    ps_t[:, k], e_t[:, mblk * 128 : (mblk + 1) * 128], identity
                )
            nc.any.tensor_copy(out=AT[:, t, g * 8 : (g + 1) * 8, :], in_=ps_t[:])


_nc_cache = {}


def _get_nc(B_local, N, M, D):
    key = (B_local, N, M, D)
    if key not in _nc_cache:
        _nc_cache[key] = build_nc(B_local, N, M, D)
    return _nc_cache[key]


def kernel(u, s):
    u = np.ascontiguousarray(u, dtype=np.float32)
    s = np.ascontiguousarray(s, dtype=np.float32)
    B, N, M = s.shape
    D = u.shape[2]
    assert B % N_CORES == 0
    B_local = B // N_CORES
    nc = _get_nc(B_local, N, M, D)
    in_maps = [
        {
            "s": s[i * B_local : (i + 1) * B_local],
            "u": u[i * B_local : (i + 1) * B_local],
        }
        for i in range(N_CORES)
    ]
    res = run_bass_kernel_spmd(nc, in_maps, core_ids=list(range(N_CORES)))
    return np.concatenate([r["out"] for r in res.results], axis=0)



# revision 2
# speedup vs baseline: 15.1823x; 15.1823x over previous
"""Context2Query kernel for Trainium2 (Bass/Tile), 8 NeuronCores.

Computes, for inputs u[B, M, D] (query encodings) and s[B, N, M]
(similarity logits):

    A = softmax(s, axis=-1)            # [B, N, M]
    out = einsum('bnm,bmd->bdn', A, u) # [B, D, N]

Sharding: data-parallel over batch. B=16 across 8 cores -> 2 batches/core.
Per batch, per 128-row tile of s (n on partitions):
  - ACT: E = exp(s) in bf16 with fused row-sum (accum_out)  [no max-sub
    needed: logits are N(0,1), exp range ~e^+-6 is safe in fp32]
  - DVE: rinv = 1/sum;  A = E * rinv (per-partition scalar)
  - DMA xbar transpose (SBUF->SBUF, bf16): A tile -> A^T laid out
    [m_in_block(P), t, m_block, n] so the tensor engine sees contraction
    dim m on partitions.
  - PE: out[dblk, n-chunk] += u_bf16[mblk, dblk]^T @ A^T[mblk, n-chunk]
    accumulated over 16 m-blocks in one PSUM bank. PE does only matmuls
    (stays HAM-warm).
u is cast fp32->bf16 for free during its DMA load (SWDGE cast).
"""

import numpy as np

import concourse.bacc as bacc
import concourse.mybir as mybir
import concourse.tile as tile
from concourse.bass_utils import run_bass_kernel_spmd
from concourse.masks import make_identity

F32 = mybir.dt.float32
BF16 = mybir.dt.bfloat16
P = 128

N_CORES = 8


def build_nc(
    B_local,
    N,
    M,
    D,
    n_cores=N_CORES,
    NT=512,
    repeat=1,
    tr_mode="dma",
    db_lim=None,
    u_bf16_host=False,
    skip_tr=False,
    dep_free=False,
    tr_eng="sync",
    mix_pe_ts=(1, 3),
    tr_grouped=False,
    rhs_contig=False,
    sim_trace=False,
):
    assert N % NT == 0 and M % P == 0 and D % P == 0 and NT % P == 0
    assert tr_mode in ("dma", "pe", "mix", "pe2", "hy")
    nc = bacc.Bacc("TRN2", target_bir_lowering=False, num_devices=n_cores)
    s = nc.dram_tensor("s", [B_local, N, M], F32, kind="ExternalInput").ap()
    u_dt = BF16 if u_bf16_host else F32
    u = nc.dram_tensor("u", [B_local, M, D], u_dt, kind="ExternalInput").ap()
    out = nc.dram_tensor("out", [B_local, D, N], F32, kind="ExternalOutput").ap()

    MB = M // P  # contraction blocks
    DB = D // P  # output-partition blocks
    NCH = N // NT  # n chunks
    T = NT // P  # 128-row subtiles per chunk

    with tile.TileContext(nc, trace_sim=sim_trace) as tc:
        with (
            tc.tile_pool(name="u_pool", bufs=2) as u_pool,
            tc.tile_pool(name="s_pool", bufs=4) as s_pool,
            tc.tile_pool(
                name="e_pool",
                bufs=(6 if (tr_grouped or tr_mode in ("pe2", "hy")) else 3),
            ) as e_pool,
            tc.tile_pool(name="at_pool", bufs=2) as at_pool,
            tc.tile_pool(name="o_pool", bufs=2) as o_pool,
            tc.tile_pool(name="st_pool", bufs=4 * T) as st_pool,
            tc.tile_pool(name="singles", bufs=1) as singles,
            tc.tile_pool(name="ps_pool", bufs=4, space="PSUM") as ps_pool,
            tc.tile_pool(name="pst_pool", bufs=2, space="PSUM") as pst_pool,
        ):
            identity = None
            if tr_mode in ("pe", "mix", "pe2", "hy"):
                identity = singles.tile([P, P], BF16)
                make_identity(nc, identity)
            at_const = None
            if dep_free:
                at_const = singles.tile([P, T, MB, P], BF16)
                nc.vector.memset(at_const[:], 0)
            tr_dma = nc.sync if tr_eng == "sync" else nc.scalar
            for b in [b for _ in range(repeat) for b in range(B_local)]:
                u_bf = u_pool.tile([P, MB, D], BF16)
                u_src = u[b].rearrange("(mB p) d -> p mB d", p=P)
                if u_bf16_host:
                    nc.sync.dma_start(out=u_bf[:], in_=u_src)
                else:
                    nc.gpsimd.dma_start(out=u_bf[:], in_=u_src)
                if tr_mode == "hy":
                    # Hybrid pipeline: per chunk, subtiles 0/2 transpose via
                    # DMA xbar (pair issued adjacently to amortize the
                    # xbar-mode switch), subtiles 1/3 via PE in small packets
                    # interleaved between matmul groups.
                    GRP = MB // 2
                    e_cur = {}

                    def produce_e(cc, t):
                        n0 = cc * NT + t * P
                        s_t = s_pool.tile([P, M], F32)
                        nc.sync.dma_start(out=s_t[:], in_=s[b, n0 : n0 + P, :])
                        e_t = e_pool.tile([P, M], BF16)
                        sum_t = st_pool.tile([P, 1], F32, tag="sum")
                        nc.scalar.activation(
                            out=e_t[:],
                            in_=s_t[:],
                            func=mybir.ActivationFunctionType.Exp,
                            accum_out=sum_t[:],
                        )
                        rinv = st_pool.tile([P, 1], F32, tag="rinv")
                        nc.vector.reciprocal(rinv[:], sum_t[:])
                        nc.vector.tensor_scalar_mul(e_t[:], e_t[:], rinv[:])
                        e_cur[(cc, t)] = e_t

                    def pe_tr_half(cc, t, half, AT_next):
                        e_t = e_cur[(cc, t)]
                        ps_t = pst_pool.tile([P, GRP, P], BF16)
                        for k in range(GRP):
                            mblk = half * GRP + k
                            nc.tensor.transpose(
                                ps_t[:, k], e_t[:, mblk * P : (mblk + 1) * P], identity
                            )
                        at_dst = AT_next[:, t, half * GRP : (half + 1) * GRP, :]
                        if half == 0:
                            nc.vector.tensor_copy(out=at_dst, in_=ps_t[:])
                        else:
                            nc.scalar.copy(out=at_dst, in_=ps_t[:])

                    def produce_slot(cc, slot, AT_next):
                        if slot == 0:
                            produce_e(cc, 0)
                        elif slot == 1:
                            produce_e(cc, 1)
                            pe_tr_half(cc, 1, 0, AT_next)
                        elif slot == 2:
                            pe_tr_half(cc, 1, 1, AT_next)
                        elif slot == 3:
                            produce_e(cc, 2)
                            nc.sync.dma_start_transpose(
                                AT_next[:, 0], e_cur[(cc, 0)][:]
                            )
                            nc.sync.dma_start_transpose(
                                AT_next[:, 2], e_cur[(cc, 2)][:]
                            )
                        elif slot == 4:
                            produce_e(cc, 3)
                            pe_tr_half(cc, 3, 0, AT_next)
                        elif slot == 5:
                            pe_tr_half(cc, 3, 1, AT_next)

                    NSLOT = 6
                    ATs = at_pool.tile([P, T, MB, P], BF16)
                    for slot in range(NSLOT):
                        produce_slot(0, slot, ATs)
                    for c in range(NCH):
                        AT_cur = ATs
                        if c + 1 < NCH:
                            ATs = at_pool.tile([P, T, MB, P], BF16)
                        o_t = o_pool.tile([P, DB, NT], F32)
                        for dblk in range(DB):
                            ps = ps_pool.tile([P, NT], F32)
                            for mblk in range(MB):
                                nc.tensor.matmul(
                                    ps[:],
                                    u_bf[:, mblk, dblk * P : (dblk + 1) * P],
                                    AT_cur[:, :, mblk, :],
                                    start=(mblk == 0),
                                    stop=(mblk == MB - 1),
                                )
                            if dblk % 2 == 0:
                                nc.vector.tensor_copy(out=o_t[:, dblk, :], in_=ps[:])
                            else:
                                nc.scalar.copy(out=o_t[:, dblk, :], in_=ps[:])
                            if c + 1 < NCH and dblk < NSLOT:
                                produce_slot(c + 1, dblk, ATs)
                        if c + 1 < NCH:
                            for slot in range(DB, NSLOT):
                                produce_slot(c + 1, slot, ATs)
                        nc.sync.dma_start(
                            out=out[b].rearrange("(dB p) n -> p dB n", p=P)[
                                :, :, c * NT : (c + 1) * NT
                            ],
                            in_=o_t[:],
                        )
                    continue
                if tr_mode == "pe2":
                    # Software-pipelined emission: chunk c+1's softmax +
                    # PE-transposes are emitted in small packets between
                    # chunk c's matmul groups, so PE transpose bursts stay
                    # short (HAM stays warm) and overlap naturally.
                    GRP = MB // 2
                    e_cur = {}

                    def produce_packet(cc, pkt, AT_next):
                        t, half = pkt // 2, pkt % 2
                        if half == 0:
                            n0 = cc * NT + t * P
                            s_t = s_pool.tile([P, M], F32)
                            nc.sync.dma_start(out=s_t[:], in_=s[b, n0 : n0 + P, :])
                            e_t = e_pool.tile([P, M], BF16)
                            sum_t = st_pool.tile([P, 1], F32, tag="sum")
                            nc.scalar.activation(
                                out=e_t[:],
                                in_=s_t[:],
                                func=mybir.ActivationFunctionType.Exp,
                                accum_out=sum_t[:],
                            )
                            rinv = st_pool.tile([P, 1], F32, tag="rinv")
                            nc.vector.reciprocal(rinv[:], sum_t[:])
                            nc.vector.tensor_scalar_mul(e_t[:], e_t[:], rinv[:])
                            e_cur[(cc, t)] = e_t
                        e_t = e_cur[(cc, t)]
                        ps_t = pst_pool.tile([P, GRP, P], BF16)
                        for k in range(GRP):
                            mblk = half * GRP + k
                            nc.tensor.transpose(
                                ps_t[:, k], e_t[:, mblk * P : (mblk + 1) * P], identity
                            )
                        at_dst = AT_next[:, t, half * GRP : (half + 1) * GRP, :]
                        if pkt % 2 == 0:
                            nc.vector.tensor_copy(out=at_dst, in_=ps_t[:])
                        else:
                            nc.scalar.copy(out=at_dst, in_=ps_t[:])

                    ATs = at_pool.tile([P, T, MB, P], BF16)
                    for pkt in range(2 * T):
                        produce_packet(0, pkt, ATs)
                    for c in range(NCH):
                        AT_cur = ATs
                        if c + 1 < NCH:
                            ATs = at_pool.tile([P, T, MB, P], BF16)
                        o_t = o_pool.tile([P, DB, NT], F32)
                        for dblk in range(DB):
                            ps = ps_pool.tile([P, NT], F32)
                            for mblk in range(MB):
                                nc.tensor.matmul(
                                    ps[:],
                                    u_bf[:, mblk, dblk * P : (dblk + 1) * P],
                                    AT_cur[:, :, mblk, :],
                                    start=(mblk == 0),
                                    stop=(mblk == MB - 1),
                                )
                            if dblk % 2 == 0:
                                nc.vector.tensor_copy(out=o_t[:, dblk, :], in_=ps[:])
                            else:
                                nc.scalar.copy(out=o_t[:, dblk, :], in_=ps[:])
                            if c + 1 < NCH and dblk < 2 * T:
                                produce_packet(c + 1, dblk, ATs)
                        if c + 1 < NCH:
                            for pkt in range(DB, 2 * T):
                                produce_packet(c + 1, pkt, ATs)
                        nc.sync.dma_start(
                            out=out[b].rearrange("(dB p) n -> p dB n", p=P)[
                                :, :, c * NT : (c + 1) * NT
                            ],
                            in_=o_t[:],
                        )
                    continue
                for c in range(NCH):
                    AT = at_pool.tile([P, T, MB, P], BF16)
                    if skip_tr:
                        nc.vector.memset(AT[:], 0)
                    e_tiles = {}
                    for t in range(T):
                        if skip_tr:
                            break
                        n0 = c * NT + t * P
                        s_t = s_pool.tile([P, M], F32)
                        nc.sync.dma_start(out=s_t[:], in_=s[b, n0 : n0 + P, :])
                        e_t = e_pool.tile([P, M], BF16)
                        sum_t = st_pool.tile([P, 1], F32, tag="sum")
                        nc.scalar.activation(
                            out=e_t[:],
                            in_=s_t[:],
                            func=mybir.ActivationFunctionType.Exp,
                            accum_out=sum_t[:],
                        )
                        rinv = st_pool.tile([P, 1], F32, tag="rinv")
                        nc.vector.reciprocal(rinv[:], sum_t[:])
                        nc.vector.tensor_scalar_mul(e_t[:], e_t[:], rinv[:])
                        if tr_grouped:
                            e_tiles[t] = e_t
                            continue
                        _emit_transpose(
                            nc, tr_mode, tr_dma, mix_pe_ts, AT, t, e_t,
                            identity, pst_pool, MB,
                        )
                    if tr_grouped and not skip_tr:
                        for t in range(T):
                            _emit_transpose(
                                nc, tr_mode, tr_dma, mix_pe_ts, AT, t,
                                e_tiles[t], identity, pst_pool, MB,
                            )
                    o_t = o_pool.tile([P, DB, NT], F32)
                    for dblk in range(DB if db_lim is None else db_lim):
                        ps = ps_pool.tile([P, NT], F32)
                        rhs_src = at_const if dep_free else AT
                        for mblk in range(MB):
                            if rhs_contig:
                                rhs = rhs_src[:, mblk % T, 0 : NT // P, :]
                            else:
                                rhs = rhs_src[:, :, mblk, :]
                            nc.tensor.matmul(
                                ps[:],
                                u_bf[:, mblk, dblk * P : (dblk + 1) * P],
                                rhs,
                                start=(mblk == 0),
                                stop=(mblk == MB - 1),
                            )
                        nc.any.tensor_copy(out=o_t[:, dblk, :], in_=ps[:])
                    nc.sync.dma_start(
                        out=out[b].rearrange("(dB p) n -> p dB n", p=P)[
                            :, :, c * NT : (c + 1) * NT
                        ],
                        in_=o_t[:],
                    )
    nc.compile()
    return nc


def _emit_transpose(nc, tr_mode, tr_dma, mix_pe_ts, AT, t, e_t, identity, pst_pool, MB):
    use_pe = tr_mode == "pe" or (tr_mode == "mix" and t in mix_pe_ts)
    if not use_pe:
        tr_dma.dma_start_transpose(AT[:, t], e_t[:])
    else:
        # PE transpose in groups of 8 m-blocks per PSUM bank
        # (bf16: 8*128 = 1024 elems = 2 KiB)
        for g in range(MB // 8):
            ps_t = pst_pool.tile([128, 8, 128], BF16)
            for k in range(8):
                mblk = g * 8 + k
                nc.tensor.transpose(
                    ps_t[:, k], e_t[:, mblk * 128 : (mblk + 1) * 128], identity
                )
            nc.any.tensor_copy(out=AT[:, t, g * 8 : (g + 1) * 8, :], in_=ps_t[:])


_nc_cache = {}


def _get_nc(B_local, N, M, D, **kwargs):
    key = (B_local, N, M, D, tuple(sorted(kwargs.items())))
    if key not in _nc_cache:
        _nc_cache[key] = build_nc(B_local, N, M, D, **kwargs)
    return _nc_cache[key]


_runner_cache = {}


def _get_runner(nc, n_cores=N_CORES):
    """Persistent jitted shard_map executor for `nc`.

    run_bass_kernel_spmd rebuilds the jit closure (full retrace + XLA
    compile), re-concatenates the full inputs on host, and reallocates +
    re-transfers zero output-donation buffers on EVERY call. Building the
    executor once and keeping the zero output params device-resident makes
    repeat kernel() calls transfer-bound only.
    """
    if id(nc) in _runner_cache:
        return _runner_cache[id(nc)]

    import jax
    import concourse.mybir as mybir
    from concourse import bass2jax
    from jax.sharding import Mesh, NamedSharding, PartitionSpec
    from jax.experimental.shard_map import shard_map

    bass2jax.install_neuronx_cc_hook()

    partition_name = nc.partition_id_tensor.name if nc.partition_id_tensor else None
    in_names, out_names, out_avals = [], [], []
    for alloc in nc.m.functions[0].allocations:
        if not isinstance(alloc, mybir.MemoryLocationSet):
            continue
        name = alloc.memorylocations[0].name
        if alloc.kind == "ExternalInput":
            if name != partition_name:
                in_names.append(name)
        elif alloc.kind == "ExternalOutput":
            out_names.append(name)
            shape = tuple(alloc.tensor_shape)
            dtype = mybir.dt.np(alloc.dtype)
            out_avals.append(jax.core.ShapedArray(shape, dtype))
    all_in_names = list(in_names) + list(out_names)
    if partition_name is not None:
        all_in_names.append(partition_name)

    def _body(*args):
        operands = list(args)
        if partition_name is not None:
            operands.append(bass2jax.partition_id_tensor())
        outs = bass2jax._bass_exec_p.bind(
            *operands,
            out_avals=tuple(out_avals),
            in_names=tuple(all_in_names),
            out_names=tuple(out_names),
            lowering_input_output_aliases=(),
            sim_require_finite=True,
            sim_require_nnan=True,
            nc=nc,
        )
        return tuple(outs)

    devices = jax.devices()[:n_cores]
    mesh = Mesh(np.asarray(devices), ("core",))
    sharding = NamedSharding(mesh, PartitionSpec("core"))
    in_specs = (PartitionSpec("core"),) * (len(in_names) + len(out_names))
    out_specs = (PartitionSpec("core"),) * len(out_names)
    sharded = jax.jit(
        shard_map(
            _body, mesh=mesh, in_specs=in_specs, out_specs=out_specs, check_rep=False
        ),
        keep_unused=True,
    )
    # The NEFF's output tensors are bound to these input params; the kernel
    # writes every element, so contents are irrelevant — keep device-resident.
    dev_zeros = [
        jax.device_put(
            np.zeros((n_cores * a.shape[0], *a.shape[1:]), a.dtype), sharding
        )
        for a in out_avals
    ]
    runner = (sharded, sharding, in_names, dev_zeros)
    _runner_cache[id(nc)] = runner
    return runner


def kernel(u, s):
    import jax

    u = np.ascontiguousarray(u, dtype=np.float32)
    s = np.ascontiguousarray(s, dtype=np.float32)
    B, N, M = s.shape
    D = u.shape[2]
    assert B % N_CORES == 0
    B_local = B // N_CORES
    nc = _get_nc(B_local, N, M, D)
    sharded, sharding, in_names, dev_zeros = _get_runner(nc)
    # Full arrays sharded on batch dim 0 — each core gets its B_local slice.
    host_in = {"s": s, "u": u}
    dev_in = [jax.device_put(host_in[name], sharding) for name in in_names]
    out = sharded(*dev_in, *dev_zeros)
    return np.asarray(out[0])



# revision 19
# speedup vs baseline: 16.2920x; 1.0731x over previous
"""Context2Query kernel for Trainium2 (Bass/Tile), 8 NeuronCores.

Computes, for inputs u[B, M, D] (query encodings) and s[B, N, M]
(similarity logits):

    A = softmax(s, axis=-1)            # [B, N, M]
    out = einsum('bnm,bmd->bdn', A, u) # [B, D, N]

Sharding: data-parallel over batch. B=16 across 8 cores -> 2 batches/core.
Per batch, per 128-row tile of s (n on partitions):
  - ACT: E = exp(s) in bf16 with fused row-sum (accum_out)  [no max-sub
    needed: logits are N(0,1), exp range ~e^+-6 is safe in fp32]
  - DVE: rinv = 1/sum;  A = E * rinv (per-partition scalar)
  - DMA xbar transpose (SBUF->SBUF, bf16): A tile -> A^T laid out
    [m_in_block(P), t, m_block, n] so the tensor engine sees contraction
    dim m on partitions.
  - PE: out[dblk, n-chunk] += u_bf16[mblk, dblk]^T @ A^T[mblk, n-chunk]
    accumulated over 16 m-blocks in one PSUM bank. PE does only matmuls
    (stays HAM-warm).
u is cast fp32->bf16 for free during its DMA load (SWDGE cast).
"""

import numpy as np

import concourse.bacc as bacc
import concourse.mybir as mybir
import concourse.tile as tile
from concourse.bass_utils import run_bass_kernel_spmd
from concourse.masks import make_identity

F32 = mybir.dt.float32
BF16 = mybir.dt.bfloat16
P = 128

N_CORES = 8


def build_nc(
    B_local,
    N,
    M,
    D,
    n_cores=N_CORES,
    NT=512,
    repeat=1,
    tr_mode="dma",
    db_lim=None,
    u_bf16_host=False,
    skip_tr=False,
    dep_free=False,
    tr_eng="sync",
    mix_pe_ts=(1, 3),
    tr_grouped=False,
    rhs_contig=False,
    sim_trace=False,
    out_eng="sync",
    copy_eng="any",
    s_eng="sync",
    s_bf16_host=False,
    at_bufs=2,
    s_bufs=4,
    e_bufs=None,
    u_bufs=2,
    defer_norm=False,
):
    assert N % NT == 0 and M % P == 0 and D % P == 0 and NT % P == 0
    assert tr_mode in ("dma", "pe", "mix", "pe2", "hy")
    nc = bacc.Bacc("TRN2", target_bir_lowering=False, num_devices=n_cores)
    s_dt = BF16 if s_bf16_host else F32
    s = nc.dram_tensor("s", [B_local, N, M], s_dt, kind="ExternalInput").ap()
    u_dt = BF16 if u_bf16_host else F32
    u = nc.dram_tensor("u", [B_local, M, D], u_dt, kind="ExternalInput").ap()
    out = nc.dram_tensor("out", [B_local, D, N], F32, kind="ExternalOutput").ap()

    MB = M // P  # contraction blocks
    DB = D // P  # output-partition blocks
    NCH = N // NT  # n chunks
    T = NT // P  # 128-row subtiles per chunk

    with tile.TileContext(nc, trace_sim=sim_trace) as tc:
        with (
            tc.tile_pool(name="u_pool", bufs=u_bufs) as u_pool,
            tc.tile_pool(name="s_pool", bufs=s_bufs) as s_pool,
            tc.tile_pool(
                name="e_pool",
                bufs=(
                    e_bufs
                    if e_bufs is not None
                    else (6 if (tr_grouped or tr_mode in ("pe2", "hy")) else 3)
                ),
            ) as e_pool,
            tc.tile_pool(name="at_pool", bufs=at_bufs) as at_pool,
            tc.tile_pool(name="o_pool", bufs=2) as o_pool,
            tc.tile_pool(name="st_pool", bufs=4 * T) as st_pool,
            tc.tile_pool(name="singles", bufs=1) as singles,
            tc.tile_pool(name="ps_pool", bufs=4, space="PSUM") as ps_pool,
            tc.tile_pool(name="pst_pool", bufs=2, space="PSUM") as pst_pool,
            tc.tile_pool(name="sums_pool", bufs=4) as sums_pool,
            tc.tile_pool(name="rb_pool", bufs=2) as rb_pool,
            tc.tile_pool(name="rbp_pool", bufs=2, space="PSUM") as rbp_pool,
        ):
            identity = None
            if tr_mode in ("pe", "mix", "pe2", "hy"):
                identity = singles.tile([P, P], BF16)
                make_identity(nc, identity)
            identity_f32 = None
            ones_row = None
            if defer_norm:
                identity_f32 = singles.tile([P, P], F32)
                make_identity(nc, identity_f32)
                ones_row = singles.tile([1, P], F32)
                nc.vector.memset(ones_row[:], 1.0)
            at_const = None
            if dep_free:
                at_const = singles.tile([P, T, MB, P], BF16)
                nc.vector.memset(at_const[:], 0)
            tr_dma = nc.sync if tr_eng == "sync" else nc.scalar
            eng_map = {"sync": nc.sync, "scalar": nc.scalar, "gpsimd": nc.gpsimd}
            out_dma = eng_map[out_eng]
            for b in [b for _ in range(repeat) for b in range(B_local)]:
                u_bf = u_pool.tile([P, MB, D], BF16)
                u_src = u[b].rearrange("(mB p) d -> p mB d", p=P)
                if u_bf16_host:
                    nc.sync.dma_start(out=u_bf[:], in_=u_src)
                else:
                    nc.gpsimd.dma_start(out=u_bf[:], in_=u_src)
                if tr_mode == "hy":
                    # Hybrid pipeline: per chunk, subtiles 0/2 transpose via
                    # DMA xbar (pair issued adjacently to amortize the
                    # xbar-mode switch), subtiles 1/3 via PE in small packets
                    # interleaved between matmul groups.
                    GRP = MB // 2
                    e_cur = {}

                    def produce_e(cc, t):
                        n0 = cc * NT + t * P
                        s_t = s_pool.tile([P, M], F32)
                        nc.sync.dma_start(out=s_t[:], in_=s[b, n0 : n0 + P, :])
                        e_t = e_pool.tile([P, M], BF16)
                        sum_t = st_pool.tile([P, 1], F32, tag="sum")
                        nc.scalar.activation(
                            out=e_t[:],
                            in_=s_t[:],
                            func=mybir.ActivationFunctionType.Exp,
                            accum_out=sum_t[:],
                        )
                        rinv = st_pool.tile([P, 1], F32, tag="rinv")
                        nc.vector.reciprocal(rinv[:], sum_t[:])
                        nc.vector.tensor_scalar_mul(e_t[:], e_t[:], rinv[:])
                        e_cur[(cc, t)] = e_t

                    def pe_tr_half(cc, t, half, AT_next):
                        e_t = e_cur[(cc, t)]
                        ps_t = pst_pool.tile([P, GRP, P], BF16)
                        for k in range(GRP):
                            mblk = half * GRP + k
                            nc.tensor.transpose(
                                ps_t[:, k], e_t[:, mblk * P : (mblk + 1) * P], identity
                            )
                        at_dst = AT_next[:, t, half * GRP : (half + 1) * GRP, :]
                        if half == 0:
                            nc.vector.tensor_copy(out=at_dst, in_=ps_t[:])
                        else:
                            nc.scalar.copy(out=at_dst, in_=ps_t[:])

                    def produce_slot(cc, slot, AT_next):
                        if slot == 0:
                            produce_e(cc, 0)
                        elif slot == 1:
                            produce_e(cc, 1)
                            pe_tr_half(cc, 1, 0, AT_next)
                        elif slot == 2:
                            pe_tr_half(cc, 1, 1, AT_next)
                        elif slot == 3:
                            produce_e(cc, 2)
                            nc.sync.dma_start_transpose(
                                AT_next[:, 0], e_cur[(cc, 0)][:]
                            )
                            nc.sync.dma_start_transpose(
                                AT_next[:, 2], e_cur[(cc, 2)][:]
                            )
                        elif slot == 4:
                            produce_e(cc, 3)
                            pe_tr_half(cc, 3, 0, AT_next)
                        elif slot == 5:
                            pe_tr_half(cc, 3, 1, AT_next)

                    NSLOT = 6
                    ATs = at_pool.tile([P, T, MB, P], BF16)
                    for slot in range(NSLOT):
                        produce_slot(0, slot, ATs)
                    for c in range(NCH):
                        AT_cur = ATs
                        if c + 1 < NCH:
                            ATs = at_pool.tile([P, T, MB, P], BF16)
                        o_t = o_pool.tile([P, DB, NT], F32)
                        for dblk in range(DB):
                            ps = ps_pool.tile([P, NT], F32)
                            for mblk in range(MB):
                                nc.tensor.matmul(
                                    ps[:],
                                    u_bf[:, mblk, dblk * P : (dblk + 1) * P],
                                    AT_cur[:, :, mblk, :],
                                    start=(mblk == 0),
                                    stop=(mblk == MB - 1),
                                )
                            if dblk % 2 == 0:
                                nc.vector.tensor_copy(out=o_t[:, dblk, :], in_=ps[:])
                            else:
                                nc.scalar.copy(out=o_t[:, dblk, :], in_=ps[:])
                            if c + 1 < NCH and dblk < NSLOT:
                                produce_slot(c + 1, dblk, ATs)
                        if c + 1 < NCH:
                            for slot in range(DB, NSLOT):
                                produce_slot(c + 1, slot, ATs)
                        nc.sync.dma_start(
                            out=out[b].rearrange("(dB p) n -> p dB n", p=P)[
                                :, :, c * NT : (c + 1) * NT
                            ],
                            in_=o_t[:],
                        )
                    continue
                if tr_mode == "pe2":
                    # Software-pipelined emission: chunk c+1's softmax +
                    # PE-transposes are emitted in small packets between
                    # chunk c's matmul groups, so PE transpose bursts stay
                    # short (HAM stays warm) and overlap naturally.
                    GRP = MB // 2
                    e_cur = {}

                    def produce_packet(cc, pkt, AT_next):
                        t, half = pkt // 2, pkt % 2
                        if half == 0:
                            n0 = cc * NT + t * P
                            s_t = s_pool.tile([P, M], s_dt)
                            nc.sync.dma_start(out=s_t[:], in_=s[b, n0 : n0 + P, :])
                            e_t = e_pool.tile([P, M], BF16)
                            sum_t = st_pool.tile([P, 1], F32, tag="sum")
                            nc.scalar.activation(
                                out=e_t[:],
                                in_=s_t[:],
                                func=mybir.ActivationFunctionType.Exp,
                                accum_out=sum_t[:],
                            )
                            rinv = st_pool.tile([P, 1], F32, tag="rinv")
                            nc.vector.reciprocal(rinv[:], sum_t[:])
                            nc.vector.tensor_scalar_mul(e_t[:], e_t[:], rinv[:])
                            e_cur[(cc, t)] = e_t
                        e_t = e_cur[(cc, t)]
                        ps_t = pst_pool.tile([P, GRP, P], BF16)
                        for k in range(GRP):
                            mblk = half * GRP + k
                            nc.tensor.transpose(
                                ps_t[:, k], e_t[:, mblk * P : (mblk + 1) * P], identity
                            )
                        at_dst = AT_next[:, t, half * GRP : (half + 1) * GRP, :]
                        if pkt % 2 == 0:
                            nc.vector.tensor_copy(out=at_dst, in_=ps_t[:])
                        else:
                            nc.scalar.copy(out=at_dst, in_=ps_t[:])

                    ATs = at_pool.tile([P, T, MB, P], BF16)
                    for pkt in range(2 * T):
                        produce_packet(0, pkt, ATs)
                    for c in range(NCH):
                        AT_cur = ATs
                        if c + 1 < NCH:
                            ATs = at_pool.tile([P, T, MB, P], BF16)
                        o_t = o_pool.tile([P, DB, NT], F32)
                        for dblk in range(DB):
                            ps = ps_pool.tile([P, NT], F32)
                            for mblk in range(MB):
                                nc.tensor.matmul(
                                    ps[:],
                                    u_bf[:, mblk, dblk * P : (dblk + 1) * P],
                                    AT_cur[:, :, mblk, :],
                                    start=(mblk == 0),
                                    stop=(mblk == MB - 1),
                                )
                            if dblk % 2 == 0:
                                nc.vector.tensor_copy(out=o_t[:, dblk, :], in_=ps[:])
                            else:
                                nc.scalar.copy(out=o_t[:, dblk, :], in_=ps[:])
                            if c + 1 < NCH and dblk < 2 * T:
                                produce_packet(c + 1, dblk, ATs)
                        if c + 1 < NCH:
                            for pkt in range(DB, 2 * T):
                                produce_packet(c + 1, pkt, ATs)
                        nc.sync.dma_start(
                            out=out[b].rearrange("(dB p) n -> p dB n", p=P)[
                                :, :, c * NT : (c + 1) * NT
                            ],
                            in_=o_t[:],
                        )
                    continue
                for c in range(NCH):
                    AT = at_pool.tile([P, T, MB, P], BF16)
                    if skip_tr:
                        nc.vector.memset(AT[:], 0)
                    e_tiles = {}
                    sums = None
                    if defer_norm and not skip_tr:
                        sums = sums_pool.tile([P, T], F32)
                    for t in range(T):
                        if skip_tr:
                            break
                        n0 = c * NT + t * P
                        s_t = s_pool.tile([P, M], s_dt)
                        s_dma = (
                            eng_map[s_eng]
                            if s_eng != "alt"
                            else (nc.sync if t % 2 == 0 else nc.scalar)
                        )
                        s_dma.dma_start(out=s_t[:], in_=s[b, n0 : n0 + P, :])
                        e_t = e_pool.tile([P, M], BF16)
                        if defer_norm:
                            nc.scalar.activation(
                                out=e_t[:],
                                in_=s_t[:],
                                func=mybir.ActivationFunctionType.Exp,
                                accum_out=sums[:, t : t + 1],
                            )
                        else:
                            sum_t = st_pool.tile([P, 1], F32, tag="sum")
                            nc.scalar.activation(
                                out=e_t[:],
                                in_=s_t[:],
                                func=mybir.ActivationFunctionType.Exp,
                                accum_out=sum_t[:],
                            )
                            rinv = st_pool.tile([P, 1], F32, tag="rinv")
                            nc.vector.reciprocal(rinv[:], sum_t[:])
                            nc.vector.tensor_scalar_mul(e_t[:], e_t[:], rinv[:])
                        if tr_grouped:
                            e_tiles[t] = e_t
                            continue
                        _emit_transpose(
                            nc, tr_mode, tr_dma, mix_pe_ts, AT, t, e_t,
                            identity, pst_pool, MB,
                        )
                    if tr_grouped and not skip_tr:
                        for t in range(T):
                            _emit_transpose(
                                nc, tr_mode, tr_dma, mix_pe_ts, AT, t,
                                e_tiles[t], identity, pst_pool, MB,
                            )
                    rb = None
                    if defer_norm and not skip_tr:
                        # rinv[n] broadcast to all partitions: [P,T] recip ->
                        # PE transpose -> [T,P] -> 4 rank-1 PE broadcasts ->
                        # [P, NT] in PSUM -> SBUF
                        rinv_pt = sums_pool.tile([P, T], F32)
                        nc.vector.reciprocal(rinv_pt[:], sums[:])
                        rb_ps = rbp_pool.tile([P, NT], F32)
                        for t in range(T):
                            rt_ps = rbp_pool.tile([1, P], F32)
                            nc.tensor.transpose(
                                rt_ps[:], rinv_pt[:, t : t + 1], identity_f32
                            )
                            rr_t = sums_pool.tile([1, P], F32, tag=f"rr{t}")
                            nc.vector.tensor_copy(out=rr_t[:], in_=rt_ps[:])
                            nc.tensor.matmul(
                                rb_ps[:, t * P : (t + 1) * P],
                                ones_row[:],
                                rr_t[:],
                                start=True,
                                stop=True,
                            )
                        rb = rb_pool.tile([P, NT], F32)
                        nc.vector.tensor_copy(out=rb[:], in_=rb_ps[:])
                    out_view = out[b].rearrange("(dB p) n -> p dB n", p=P)
                    o_t = None
                    if copy_eng != "psum_dma":
                        o_t = o_pool.tile([P, DB, NT], F32)
                    for dblk in range(DB if db_lim is None else db_lim):
                        ps = ps_pool.tile([P, NT], F32)
                        rhs_src = at_const if dep_free else AT
                        for mblk in range(MB):
                            if rhs_contig:
                                rhs = rhs_src[:, mblk % T, 0 : NT // P, :]
                            else:
                                rhs = rhs_src[:, :, mblk, :]
                            nc.tensor.matmul(
                                ps[:],
                                u_bf[:, mblk, dblk * P : (dblk + 1) * P],
                                rhs,
                                start=(mblk == 0),
                                stop=(mblk == MB - 1),
                            )
                        if copy_eng == "psum_dma":
                            out_dma.dma_start(
                                out=out_view[:, dblk, c * NT : (c + 1) * NT],
                                in_=ps[:],
                            )
                        elif defer_norm and not skip_tr:
                            nc.vector.scalar_tensor_tensor(
                                out=o_t[:, dblk, :],
                                in0=ps[:],
                                scalar=1.0,
                                in1=rb[:],
                                op0=mybir.AluOpType.mult,
                                op1=mybir.AluOpType.mult,
                            )
                        elif copy_eng == "dve":
                            nc.vector.tensor_copy(out=o_t[:, dblk, :], in_=ps[:])
                        else:
                            nc.any.tensor_copy(out=o_t[:, dblk, :], in_=ps[:])
                    if copy_eng != "psum_dma":
                        out_dma.dma_start(
                            out=out_view[:, :, c * NT : (c + 1) * NT],
                            in_=o_t[:],
                        )
    nc.compile()
    return nc


def _emit_transpose(nc, tr_mode, tr_dma, mix_pe_ts, AT, t, e_t, identity, pst_pool, MB):
    use_pe = tr_mode == "pe" or (tr_mode == "mix" and t in mix_pe_ts)
    if not use_pe:
        tr_dma.dma_start_transpose(AT[:, t], e_t[:])
    else:
        # PE transpose in groups of 8 m-blocks per PSUM bank
        # (bf16: 8*128 = 1024 elems = 2 KiB)
        for g in range(MB // 8):
            ps_t = pst_pool.tile([128, 8, 128], BF16)
            for k in range(8):
                mblk = g * 8 + k
                nc.tensor.transpose(
                    ps_t[:, k], e_t[:, mblk * 128 : (mblk + 1) * 128], identity
                )
            nc.any.tensor_copy(out=AT[:, t, g * 8 : (g + 1) * 8, :], in_=ps_t[:])


# Selected variant (A/B'd on hardware via repeat-differencing):
# - defer_norm: keep E=exp(s) unnormalized through the transpose + matmul,
#   fold the 1/rowsum into the PSUM->SBUF output copy (saves 16 MiB/batch of
#   SBUF traffic + shortens the A^T critical path). Also slightly MORE
#   accurate than normalizing E in bf16 (rel err 2.4e-3 vs 2.9e-3).
# - s_bf16_host: s is cast to bf16 on host; halves s HBM/DMA/SBUF traffic.
#   rel err 4.3e-3 (gate is 2e-2).
DEFAULT_CFG = {"defer_norm": True, "s_bf16_host": True}

_nc_cache = {}


def _get_nc(B_local, N, M, D, **kwargs):
    key = (B_local, N, M, D, tuple(sorted(kwargs.items())))
    if key not in _nc_cache:
        _nc_cache[key] = build_nc(B_local, N, M, D, **kwargs)
    return _nc_cache[key]


def _prep_inputs(u, s, cfg=None):
    """Cast full inputs to the dtypes the selected kernel variant declares."""
    cfg = DEFAULT_CFG if cfg is None else cfg
    u = np.ascontiguousarray(u, dtype=np.float32)
    s = np.ascontiguousarray(s, dtype=np.float32)
    if cfg.get("u_bf16_host") or cfg.get("s_bf16_host"):
        import ml_dtypes

        if cfg.get("u_bf16_host"):
            u = u.astype(ml_dtypes.bfloat16)
        if cfg.get("s_bf16_host"):
            s = s.astype(ml_dtypes.bfloat16)
    return {"u": u, "s": s}


_runner_cache = {}


def _get_runner(nc, n_cores=N_CORES):
    """Persistent jitted shard_map executor for `nc`.

    run_bass_kernel_spmd rebuilds the jit closure (full retrace + XLA
    compile), re-concatenates the full inputs on host, and reallocates +
    re-transfers zero output-donation buffers on EVERY call. Building the
    executor once and keeping the zero output params device-resident makes
    repeat kernel() calls transfer-bound only.
    """
    if id(nc) in _runner_cache:
        return _runner_cache[id(nc)]

    import jax
    import concourse.mybir as mybir
    from concourse import bass2jax
    from jax.sharding import Mesh, NamedSharding, PartitionSpec
    from jax.experimental.shard_map import shard_map

    bass2jax.install_neuronx_cc_hook()

    partition_name = nc.partition_id_tensor.name if nc.partition_id_tensor else None
    in_names, out_names, out_avals = [], [], []
    for alloc in nc.m.functions[0].allocations:
        if not isinstance(alloc, mybir.MemoryLocationSet):
            continue
        name = alloc.memorylocations[0].name
        if alloc.kind == "ExternalInput":
            if name != partition_name:
                in_names.append(name)
        elif alloc.kind == "ExternalOutput":
            out_names.append(name)
            shape = tuple(alloc.tensor_shape)
            dtype = mybir.dt.np(alloc.dtype)
            out_avals.append(jax.core.ShapedArray(shape, dtype))
    all_in_names = list(in_names) + list(out_names)
    if partition_name is not None:
        all_in_names.append(partition_name)

    def _body(*args):
        operands = list(args)
        if partition_name is not None:
            operands.append(bass2jax.partition_id_tensor())
        outs = bass2jax._bass_exec_p.bind(
            *operands,
            out_avals=tuple(out_avals),
            in_names=tuple(all_in_names),
            out_names=tuple(out_names),
            lowering_input_output_aliases=(),
            sim_require_finite=True,
            sim_require_nnan=True,
            nc=nc,
        )
        return tuple(outs)

    devices = jax.devices()[:n_cores]
    mesh = Mesh(np.asarray(devices), ("core",))
    sharding = NamedSharding(mesh, PartitionSpec("core"))
    in_specs = (PartitionSpec("core"),) * (len(in_names) + len(out_names))
    out_specs = (PartitionSpec("core"),) * len(out_names)
    sharded = jax.jit(
        shard_map(
            _body, mesh=mesh, in_specs=in_specs, out_specs=out_specs, check_rep=False
        ),
        keep_unused=True,
    )
    # The NEFF's output tensors are bound to these input params; the kernel
    # writes every element, so contents are irrelevant — keep device-resident.
    dev_zeros = [
        jax.device_put(
            np.zeros((n_cores * a.shape[0], *a.shape[1:]), a.dtype), sharding
        )
        for a in out_avals
    ]
    runner = (sharded, sharding, in_names, dev_zeros)
    _runner_cache[id(nc)] = runner
    return runner


def kernel(u, s):
    import jax

    B, N, M = s.shape
    D = u.shape[2]
    assert B % N_CORES == 0
    B_local = B // N_CORES
    nc = _get_nc(B_local, N, M, D, **DEFAULT_CFG)
    sharded, sharding, in_names, dev_zeros = _get_runner(nc)
    # Full arrays sharded on batch dim 0 — each core gets its B_local slice.
    host_in = _prep_inputs(u, s)
    dev_in = [jax.device_put(host_in[name], sharding) for name in in_names]
    out = sharded(*dev_in, *dev_zeros)
    return np.asarray(out[0])



# revision 23
# speedup vs baseline: 16.6963x; 1.0248x over previous
"""Context2Query kernel for Trainium2 (Bass/Tile), 8 NeuronCores.

Computes, for inputs u[B, M, D] (query encodings) and s[B, N, M]
(similarity logits):

    A = softmax(s, axis=-1)            # [B, N, M]
    out = einsum('bnm,bmd->bdn', A, u) # [B, D, N]

Sharding: data-parallel over batch. B=16 across 8 cores -> 2 batches/core.
Per batch, per 128-row tile of s (n on partitions):
  - ACT: E = exp(s) in bf16 with fused row-sum (accum_out)  [no max-sub
    needed: logits are N(0,1), exp range ~e^+-6 is safe in fp32]
  - DVE: rinv = 1/sum;  A = E * rinv (per-partition scalar)
  - DMA xbar transpose (SBUF->SBUF, bf16): A tile -> A^T laid out
    [m_in_block(P), t, m_block, n] so the tensor engine sees contraction
    dim m on partitions.
  - PE: out[dblk, n-chunk] += u_bf16[mblk, dblk]^T @ A^T[mblk, n-chunk]
    accumulated over 16 m-blocks in one PSUM bank. PE does only matmuls
    (stays HAM-warm).
u is cast fp32->bf16 for free during its DMA load (SWDGE cast).
"""

import numpy as np

import concourse.bacc as bacc
import concourse.mybir as mybir
import concourse.tile as tile
from concourse.bass_utils import run_bass_kernel_spmd
from concourse.masks import make_identity

F32 = mybir.dt.float32
BF16 = mybir.dt.bfloat16
P = 128

N_CORES = 8


def build_nc(
    B_local,
    N,
    M,
    D,
    n_cores=N_CORES,
    NT=512,
    repeat=1,
    tr_mode="dma",
    db_lim=None,
    u_bf16_host=False,
    skip_tr=False,
    dep_free=False,
    tr_eng="sync",
    mix_pe_ts=(1, 3),
    tr_grouped=False,
    rhs_contig=False,
    sim_trace=False,
    out_eng="sync",
    copy_eng="any",
    s_eng="sync",
    s_bf16_host=False,
    at_bufs=2,
    s_bufs=4,
    e_bufs=None,
    u_bufs=2,
    defer_norm=False,
):
    assert N % NT == 0 and M % P == 0 and D % P == 0 and NT % P == 0
    assert tr_mode in ("dma", "pe", "mix", "pe2", "hy")
    nc = bacc.Bacc("TRN2", target_bir_lowering=False, num_devices=n_cores)
    s_dt = BF16 if s_bf16_host else F32
    s = nc.dram_tensor("s", [B_local, N, M], s_dt, kind="ExternalInput").ap()
    u_dt = BF16 if u_bf16_host else F32
    u = nc.dram_tensor("u", [B_local, M, D], u_dt, kind="ExternalInput").ap()
    out = nc.dram_tensor("out", [B_local, D, N], F32, kind="ExternalOutput").ap()

    MB = M // P  # contraction blocks
    DB = D // P  # output-partition blocks
    NCH = N // NT  # n chunks
    T = NT // P  # 128-row subtiles per chunk

    with tile.TileContext(nc, trace_sim=sim_trace) as tc:
        with (
            tc.tile_pool(name="u_pool", bufs=u_bufs) as u_pool,
            tc.tile_pool(name="s_pool", bufs=s_bufs) as s_pool,
            tc.tile_pool(
                name="e_pool",
                bufs=(
                    e_bufs
                    if e_bufs is not None
                    else (6 if (tr_grouped or tr_mode in ("pe2", "hy")) else 3)
                ),
            ) as e_pool,
            tc.tile_pool(name="at_pool", bufs=at_bufs) as at_pool,
            tc.tile_pool(name="o_pool", bufs=2) as o_pool,
            tc.tile_pool(name="st_pool", bufs=4 * T) as st_pool,
            tc.tile_pool(name="singles", bufs=1) as singles,
            tc.tile_pool(name="ps_pool", bufs=4, space="PSUM") as ps_pool,
            tc.tile_pool(name="pst_pool", bufs=2, space="PSUM") as pst_pool,
            tc.tile_pool(name="sums_pool", bufs=4) as sums_pool,
            tc.tile_pool(name="rb_pool", bufs=2) as rb_pool,
            tc.tile_pool(name="rbp_pool", bufs=1, space="PSUM") as rbp_pool,
            tc.tile_pool(name="rtp_pool", bufs=1, space="PSUM") as rtp_pool,
        ):
            identity = None
            if tr_mode in ("pe", "mix", "pe2", "hy"):
                identity = singles.tile([P, P], BF16)
                make_identity(nc, identity)
            identity_f32 = None
            ones_row = None
            if defer_norm:
                identity_f32 = singles.tile([P, P], F32)
                make_identity(nc, identity_f32)
                ones_row = singles.tile([1, P], F32)
                nc.vector.memset(ones_row[:], 1.0)
            at_const = None
            if dep_free:
                at_const = singles.tile([P, T, MB, P], BF16)
                nc.vector.memset(at_const[:], 0)
            tr_dma = nc.sync if tr_eng == "sync" else nc.scalar
            eng_map = {"sync": nc.sync, "scalar": nc.scalar, "gpsimd": nc.gpsimd}
            out_dma = eng_map[out_eng]
            for b in [b for _ in range(repeat) for b in range(B_local)]:
                u_bf = u_pool.tile([P, MB, D], BF16)
                u_src = u[b].rearrange("(mB p) d -> p mB d", p=P)
                if u_bf16_host:
                    nc.sync.dma_start(out=u_bf[:], in_=u_src)
                else:
                    nc.gpsimd.dma_start(out=u_bf[:], in_=u_src)
                if tr_mode == "hy":
                    # Hybrid pipeline: per chunk, subtiles 0/2 transpose via
                    # DMA xbar (pair issued adjacently to amortize the
                    # xbar-mode switch), subtiles 1/3 via PE in small packets
                    # interleaved between matmul groups.
                    GRP = MB // 2
                    e_cur = {}
                    sums_cur = {}
                    rb_cur = {}

                    def produce_e(cc, t):
                        if defer_norm and t == 0:
                            sums_c = sums_pool.tile([P, T], F32)
                            sums_cur[cc] = sums_c
                        n0 = cc * NT + t * P
                        s_t = s_pool.tile([P, M], s_dt)
                        nc.sync.dma_start(out=s_t[:], in_=s[b, n0 : n0 + P, :])
                        e_t = e_pool.tile([P, M], BF16)
                        if defer_norm:
                            nc.scalar.activation(
                                out=e_t[:],
                                in_=s_t[:],
                                func=mybir.ActivationFunctionType.Exp,
                                accum_out=sums_cur[cc][:, t : t + 1],
                            )
                        else:
                            sum_t = st_pool.tile([P, 1], F32, tag="sum")
                            nc.scalar.activation(
                                out=e_t[:],
                                in_=s_t[:],
                                func=mybir.ActivationFunctionType.Exp,
                                accum_out=sum_t[:],
                            )
                            rinv = st_pool.tile([P, 1], F32, tag="rinv")
                            nc.vector.reciprocal(rinv[:], sum_t[:])
                            nc.vector.tensor_scalar_mul(e_t[:], e_t[:], rinv[:])
                        e_cur[(cc, t)] = e_t

                    def build_rb(cc):
                        sums = sums_cur[cc]
                        rinv_pt = sums_pool.tile([P, T], F32)
                        nc.vector.reciprocal(rinv_pt[:], sums[:])
                        rb_ps = rbp_pool.tile([P, NT], F32)
                        for tt in range(T):
                            rt_ps = rtp_pool.tile([1, P], F32)
                            nc.tensor.transpose(
                                rt_ps[:], rinv_pt[:, tt : tt + 1], identity_f32
                            )
                            rr_t = sums_pool.tile([1, P], F32, tag=f"rr{tt}")
                            nc.vector.tensor_copy(out=rr_t[:], in_=rt_ps[:])
                            nc.tensor.matmul(
                                rb_ps[:, tt * P : (tt + 1) * P],
                                ones_row[:],
                                rr_t[:],
                                start=True,
                                stop=True,
                            )
                        rb_t = rb_pool.tile([P, NT], F32)
                        nc.vector.tensor_copy(out=rb_t[:], in_=rb_ps[:])
                        rb_cur[cc] = rb_t

                    def pe_tr_half(cc, t, half, AT_next):
                        e_t = e_cur[(cc, t)]
                        ps_t = pst_pool.tile([P, GRP, P], BF16)
                        for k in range(GRP):
                            mblk = half * GRP + k
                            nc.tensor.transpose(
                                ps_t[:, k], e_t[:, mblk * P : (mblk + 1) * P], identity
                            )
                        at_dst = AT_next[:, t, half * GRP : (half + 1) * GRP, :]
                        if half == 0:
                            nc.vector.tensor_copy(out=at_dst, in_=ps_t[:])
                        else:
                            nc.scalar.copy(out=at_dst, in_=ps_t[:])

                    def produce_slot(cc, slot, AT_next):
                        if slot == 0:
                            produce_e(cc, 0)
                        elif slot == 1:
                            produce_e(cc, 1)
                            pe_tr_half(cc, 1, 0, AT_next)
                        elif slot == 2:
                            pe_tr_half(cc, 1, 1, AT_next)
                        elif slot == 3:
                            produce_e(cc, 2)
                            nc.sync.dma_start_transpose(
                                AT_next[:, 0], e_cur[(cc, 0)][:]
                            )
                            nc.sync.dma_start_transpose(
                                AT_next[:, 2], e_cur[(cc, 2)][:]
                            )
                        elif slot == 4:
                            produce_e(cc, 3)
                            pe_tr_half(cc, 3, 0, AT_next)
                        elif slot == 5:
                            pe_tr_half(cc, 3, 1, AT_next)
                            if defer_norm:
                                build_rb(cc)

                    NSLOT = 6
                    ATs = at_pool.tile([P, T, MB, P], BF16)
                    for slot in range(NSLOT):
                        produce_slot(0, slot, ATs)
                    for c in range(NCH):
                        AT_cur = ATs
                        if c + 1 < NCH:
                            ATs = at_pool.tile([P, T, MB, P], BF16)
                        o_t = o_pool.tile([P, DB, NT], F32)
                        for dblk in range(DB):
                            ps = ps_pool.tile([P, NT], F32)
                            for mblk in range(MB):
                                nc.tensor.matmul(
                                    ps[:],
                                    u_bf[:, mblk, dblk * P : (dblk + 1) * P],
                                    AT_cur[:, :, mblk, :],
                                    start=(mblk == 0),
                                    stop=(mblk == MB - 1),
                                )
                            if defer_norm:
                                nc.vector.scalar_tensor_tensor(
                                    out=o_t[:, dblk, :],
                                    in0=ps[:],
                                    scalar=1.0,
                                    in1=rb_cur[c][:],
                                    op0=mybir.AluOpType.mult,
                                    op1=mybir.AluOpType.mult,
                                )
                            elif dblk % 2 == 0:
                                nc.vector.tensor_copy(out=o_t[:, dblk, :], in_=ps[:])
                            else:
                                nc.scalar.copy(out=o_t[:, dblk, :], in_=ps[:])
                            if c + 1 < NCH and dblk < NSLOT:
                                produce_slot(c + 1, dblk, ATs)
                        if c + 1 < NCH:
                            for slot in range(DB, NSLOT):
                                produce_slot(c + 1, slot, ATs)
                        nc.sync.dma_start(
                            out=out[b].rearrange("(dB p) n -> p dB n", p=P)[
                                :, :, c * NT : (c + 1) * NT
                            ],
                            in_=o_t[:],
                        )
                    continue
                if tr_mode == "pe2":
                    # Software-pipelined emission: chunk c+1's softmax +
                    # PE-transposes are emitted in small packets between
                    # chunk c's matmul groups, so PE transpose bursts stay
                    # short (HAM stays warm) and overlap naturally.
                    GRP = MB // 2
                    e_cur = {}
                    sums_cur = {}
                    rb_cur = {}

                    def produce_packet(cc, pkt, AT_next):
                        t, half = pkt // 2, pkt % 2
                        if half == 0:
                            if defer_norm and t == 0:
                                sums_c = sums_pool.tile([P, T], F32)
                                sums_cur[cc] = sums_c
                            n0 = cc * NT + t * P
                            s_t = s_pool.tile([P, M], s_dt)
                            nc.sync.dma_start(out=s_t[:], in_=s[b, n0 : n0 + P, :])
                            e_t = e_pool.tile([P, M], BF16)
                            if defer_norm:
                                nc.scalar.activation(
                                    out=e_t[:],
                                    in_=s_t[:],
                                    func=mybir.ActivationFunctionType.Exp,
                                    accum_out=sums_cur[cc][:, t : t + 1],
                                )
                            else:
                                sum_t = st_pool.tile([P, 1], F32, tag="sum")
                                nc.scalar.activation(
                                    out=e_t[:],
                                    in_=s_t[:],
                                    func=mybir.ActivationFunctionType.Exp,
                                    accum_out=sum_t[:],
                                )
                                rinv = st_pool.tile([P, 1], F32, tag="rinv")
                                nc.vector.reciprocal(rinv[:], sum_t[:])
                                nc.vector.tensor_scalar_mul(e_t[:], e_t[:], rinv[:])
                            e_cur[(cc, t)] = e_t
                        e_t = e_cur[(cc, t)]
                        ps_t = pst_pool.tile([P, GRP, P], BF16)
                        for k in range(GRP):
                            mblk = half * GRP + k
                            nc.tensor.transpose(
                                ps_t[:, k], e_t[:, mblk * P : (mblk + 1) * P], identity
                            )
                        at_dst = AT_next[:, t, half * GRP : (half + 1) * GRP, :]
                        if pkt % 2 == 0:
                            nc.vector.tensor_copy(out=at_dst, in_=ps_t[:])
                        else:
                            nc.scalar.copy(out=at_dst, in_=ps_t[:])
                        if defer_norm and pkt == 2 * T - 1:
                            sums = sums_cur[cc]
                            rinv_pt = sums_pool.tile([P, T], F32)
                            nc.vector.reciprocal(rinv_pt[:], sums[:])
                            rb_ps = rbp_pool.tile([P, NT], F32)
                            for tt in range(T):
                                rt_ps = rtp_pool.tile([1, P], F32)
                                nc.tensor.transpose(
                                    rt_ps[:], rinv_pt[:, tt : tt + 1], identity_f32
                                )
                                rr_t = sums_pool.tile([1, P], F32, tag=f"rr{tt}")
                                nc.vector.tensor_copy(out=rr_t[:], in_=rt_ps[:])
                                nc.tensor.matmul(
                                    rb_ps[:, tt * P : (tt + 1) * P],
                                    ones_row[:],
                                    rr_t[:],
                                    start=True,
                                    stop=True,
                                )
                            rb_t = rb_pool.tile([P, NT], F32)
                            nc.vector.tensor_copy(out=rb_t[:], in_=rb_ps[:])
                            rb_cur[cc] = rb_t

                    ATs = at_pool.tile([P, T, MB, P], BF16)
                    for pkt in range(2 * T):
                        produce_packet(0, pkt, ATs)
                    for c in range(NCH):
                        AT_cur = ATs
                        if c + 1 < NCH:
                            ATs = at_pool.tile([P, T, MB, P], BF16)
                        o_t = o_pool.tile([P, DB, NT], F32)
                        for dblk in range(DB):
                            ps = ps_pool.tile([P, NT], F32)
                            for mblk in range(MB):
                                nc.tensor.matmul(
                                    ps[:],
                                    u_bf[:, mblk, dblk * P : (dblk + 1) * P],
                                    AT_cur[:, :, mblk, :],
                                    start=(mblk == 0),
                                    stop=(mblk == MB - 1),
                                )
                            if defer_norm:
                                nc.vector.scalar_tensor_tensor(
                                    out=o_t[:, dblk, :],
                                    in0=ps[:],
                                    scalar=1.0,
                                    in1=rb_cur[c][:],
                                    op0=mybir.AluOpType.mult,
                                    op1=mybir.AluOpType.mult,
                                )
                            elif dblk % 2 == 0:
                                nc.vector.tensor_copy(out=o_t[:, dblk, :], in_=ps[:])
                            else:
                                nc.scalar.copy(out=o_t[:, dblk, :], in_=ps[:])
                            if c + 1 < NCH and dblk < 2 * T:
                                produce_packet(c + 1, dblk, ATs)
                        if c + 1 < NCH:
                            for pkt in range(DB, 2 * T):
                                produce_packet(c + 1, pkt, ATs)
                        nc.sync.dma_start(
                            out=out[b].rearrange("(dB p) n -> p dB n", p=P)[
                                :, :, c * NT : (c + 1) * NT
                            ],
                            in_=o_t[:],
                        )
                    continue
                for c in range(NCH):
                    AT = at_pool.tile([P, T, MB, P], BF16)
                    if skip_tr:
                        nc.vector.memset(AT[:], 0)
                    e_tiles = {}
                    sums = None
                    if defer_norm and not skip_tr:
                        sums = sums_pool.tile([P, T], F32)
                    for t in range(T):
                        if skip_tr:
                            break
                        n0 = c * NT + t * P
                        s_t = s_pool.tile([P, M], s_dt)
                        s_dma = (
                            eng_map[s_eng]
                            if s_eng != "alt"
                            else (nc.sync if t % 2 == 0 else nc.scalar)
                        )
                        s_dma.dma_start(out=s_t[:], in_=s[b, n0 : n0 + P, :])
                        e_t = e_pool.tile([P, M], BF16)
                        if defer_norm:
                            nc.scalar.activation(
                                out=e_t[:],
                                in_=s_t[:],
                                func=mybir.ActivationFunctionType.Exp,
                                accum_out=sums[:, t : t + 1],
                            )
                        else:
                            sum_t = st_pool.tile([P, 1], F32, tag="sum")
                            nc.scalar.activation(
                                out=e_t[:],
                                in_=s_t[:],
                                func=mybir.ActivationFunctionType.Exp,
                                accum_out=sum_t[:],
                            )
                            rinv = st_pool.tile([P, 1], F32, tag="rinv")
                            nc.vector.reciprocal(rinv[:], sum_t[:])
                            nc.vector.tensor_scalar_mul(e_t[:], e_t[:], rinv[:])
                        if tr_grouped:
                            e_tiles[t] = e_t
                            continue
                        _emit_transpose(
                            nc, tr_mode, tr_dma, mix_pe_ts, AT, t, e_t,
                            identity, pst_pool, MB,
                        )
                    if tr_grouped and not skip_tr:
                        for t in range(T):
                            _emit_transpose(
                                nc, tr_mode, tr_dma, mix_pe_ts, AT, t,
                                e_tiles[t], identity, pst_pool, MB,
                            )
                    rb = None
                    if defer_norm and not skip_tr:
                        # rinv[n] broadcast to all partitions: [P,T] recip ->
                        # PE transpose -> [T,P] -> 4 rank-1 PE broadcasts ->
                        # [P, NT] in PSUM -> SBUF
                        rinv_pt = sums_pool.tile([P, T], F32)
                        nc.vector.reciprocal(rinv_pt[:], sums[:])
                        rb_ps = rbp_pool.tile([P, NT], F32)
                        for t in range(T):
                            rt_ps = rtp_pool.tile([1, P], F32)
                            nc.tensor.transpose(
                                rt_ps[:], rinv_pt[:, t : t + 1], identity_f32
                            )
                            rr_t = sums_pool.tile([1, P], F32, tag=f"rr{t}")
                            nc.vector.tensor_copy(out=rr_t[:], in_=rt_ps[:])
                            nc.tensor.matmul(
                                rb_ps[:, t * P : (t + 1) * P],
                                ones_row[:],
                                rr_t[:],
                                start=True,
                                stop=True,
                            )
                        rb = rb_pool.tile([P, NT], F32)
                        nc.vector.tensor_copy(out=rb[:], in_=rb_ps[:])
                    out_view = out[b].rearrange("(dB p) n -> p dB n", p=P)
                    o_t = None
                    if copy_eng != "psum_dma":
                        o_t = o_pool.tile([P, DB, NT], F32)
                    for dblk in range(DB if db_lim is None else db_lim):
                        ps = ps_pool.tile([P, NT], F32)
                        rhs_src = at_const if dep_free else AT
                        for mblk in range(MB):
                            if rhs_contig:
                                rhs = rhs_src[:, mblk % T, 0 : NT // P, :]
                            else:
                                rhs = rhs_src[:, :, mblk, :]
                            nc.tensor.matmul(
                                ps[:],
                                u_bf[:, mblk, dblk * P : (dblk + 1) * P],
                                rhs,
                                start=(mblk == 0),
                                stop=(mblk == MB - 1),
                            )
                        if copy_eng == "psum_dma":
                            out_dma.dma_start(
                                out=out_view[:, dblk, c * NT : (c + 1) * NT],
                                in_=ps[:],
                            )
                        elif defer_norm and not skip_tr:
                            nc.vector.scalar_tensor_tensor(
                                out=o_t[:, dblk, :],
                                in0=ps[:],
                                scalar=1.0,
                                in1=rb[:],
                                op0=mybir.AluOpType.mult,
                                op1=mybir.AluOpType.mult,
                            )
                        elif copy_eng == "dve":
                            nc.vector.tensor_copy(out=o_t[:, dblk, :], in_=ps[:])
                        else:
                            nc.any.tensor_copy(out=o_t[:, dblk, :], in_=ps[:])
                    if copy_eng != "psum_dma":
                        out_dma.dma_start(
                            out=out_view[:, :, c * NT : (c + 1) * NT],
                            in_=o_t[:],
                        )
    nc.compile()
    return nc


def _emit_transpose(nc, tr_mode, tr_dma, mix_pe_ts, AT, t, e_t, identity, pst_pool, MB):
    use_pe = tr_mode == "pe" or (tr_mode == "mix" and t in mix_pe_ts)
    if not use_pe:
        tr_dma.dma_start_transpose(AT[:, t], e_t[:])
    else:
        # PE transpose in groups of 8 m-blocks per PSUM bank
        # (bf16: 8*128 = 1024 elems = 2 KiB)
        for g in range(MB // 8):
            ps_t = pst_pool.tile([128, 8, 128], BF16)
            for k in range(8):
                mblk = g * 8 + k
                nc.tensor.transpose(
                    ps_t[:, k], e_t[:, mblk * 128 : (mblk + 1) * 128], identity
                )
            nc.any.tensor_copy(out=AT[:, t, g * 8 : (g + 1) * 8, :], in_=ps_t[:])


# Selected variant (A/B'd on hardware via repeat-differencing):
# - defer_norm: keep E=exp(s) unnormalized through the transpose + matmul,
#   fold the 1/rowsum into the PSUM->SBUF output copy (saves 16 MiB/batch of
#   SBUF traffic + shortens the A^T critical path). Also slightly MORE
#   accurate than normalizing E in bf16 (rel err 2.4e-3 vs 2.9e-3).
# - s_bf16_host: s is cast to bf16 on host; halves s HBM/DMA/SBUF traffic.
#   rel err 4.3e-3 (gate is 2e-2).
DEFAULT_CFG = {"defer_norm": True, "s_bf16_host": True}

_nc_cache = {}


def _get_nc(B_local, N, M, D, **kwargs):
    key = (B_local, N, M, D, tuple(sorted(kwargs.items())))
    if key not in _nc_cache:
        _nc_cache[key] = build_nc(B_local, N, M, D, **kwargs)
    return _nc_cache[key]


def _prep_inputs(u, s, cfg=None):
    """Cast full inputs to the dtypes the selected kernel variant declares."""
    cfg = DEFAULT_CFG if cfg is None else cfg
    u = np.ascontiguousarray(u, dtype=np.float32)
    s = np.ascontiguousarray(s, dtype=np.float32)
    if cfg.get("u_bf16_host") or cfg.get("s_bf16_host"):
        import ml_dtypes

        if cfg.get("u_bf16_host"):
            u = u.astype(ml_dtypes.bfloat16)
        if cfg.get("s_bf16_host"):
            s = s.astype(ml_dtypes.bfloat16)
    return {"u": u, "s": s}


_runner_cache = {}


def _get_runner(nc, n_cores=N_CORES):
    """Persistent jitted shard_map executor for `nc`.

    run_bass_kernel_spmd rebuilds the jit closure (full retrace + XLA
    compile), re-concatenates the full inputs on host, and reallocates +
    re-transfers zero output-donation buffers on EVERY call. Building the
    executor once and keeping the zero output params device-resident makes
    repeat kernel() calls transfer-bound only.
    """
    if id(nc) in _runner_cache:
        return _runner_cache[id(nc)]

    import jax
    import concourse.mybir as mybir
    from concourse import bass2jax
    from jax.sharding import Mesh, NamedSharding, PartitionSpec
    from jax.experimental.shard_map import shard_map

    bass2jax.install_neuronx_cc_hook()

    partition_name = nc.partition_id_tensor.name if nc.partition_id_tensor else None
    in_names, out_names, out_avals = [], [], []
    for alloc in nc.m.functions[0].allocations:
        if not isinstance(alloc, mybir.MemoryLocationSet):
            continue
        name = alloc.memorylocations[0].name
        if alloc.kind == "ExternalInput":
            if name != partition_name:
                in_names.append(name)
        elif alloc.kind == "ExternalOutput":
            out_names.append(name)
            shape = tuple(alloc.tensor_shape)
            dtype = mybir.dt.np(alloc.dtype)
            out_avals.append(jax.core.ShapedArray(shape, dtype))
    all_in_names = list(in_names) + list(out_names)
    if partition_name is not None:
        all_in_names.append(partition_name)

    def _body(*args):
        operands = list(args)
        if partition_name is not None:
            operands.append(bass2jax.partition_id_tensor())
        outs = bass2jax._bass_exec_p.bind(
            *operands,
            out_avals=tuple(out_avals),
            in_names=tuple(all_in_names),
            out_names=tuple(out_names),
            lowering_input_output_aliases=(),
            sim_require_finite=True,
            sim_require_nnan=True,
            nc=nc,
        )
        return tuple(outs)

    devices = jax.devices()[:n_cores]
    mesh = Mesh(np.asarray(devices), ("core",))
    sharding = NamedSharding(mesh, PartitionSpec("core"))
    in_specs = (PartitionSpec("core"),) * (len(in_names) + len(out_names))
    out_specs = (PartitionSpec("core"),) * len(out_names)
    sharded = jax.jit(
        shard_map(
            _body, mesh=mesh, in_specs=in_specs, out_specs=out_specs, check_rep=False
        ),
        keep_unused=True,
    )
    # The NEFF's output tensors are bound to these input params; the kernel
    # writes every element, so contents are irrelevant — keep device-resident.
    dev_zeros = [
        jax.device_put(
            np.zeros((n_cores * a.shape[0], *a.shape[1:]), a.dtype), sharding
        )
        for a in out_avals
    ]
    runner = (sharded, sharding, in_names, dev_zeros)
    _runner_cache[id(nc)] = runner
    return runner


def kernel(u, s):
    import jax

    B, N, M = s.shape
    D = u.shape[2]
    assert B % N_CORES == 0
    B_local = B // N_CORES
    nc = _get_nc(B_local, N, M, D, **DEFAULT_CFG)
    sharded, sharding, in_names, dev_zeros = _get_runner(nc)
    # Full arrays sharded on batch dim 0 — each core gets its B_local slice.
    host_in = _prep_inputs(u, s)
    dev_in = [jax.device_put(host_in[name], sharding) for name in in_names]
    out = sharded(*dev_in, *dev_zeros)
    return np.asarray(out[0])

